# revision 10
# baseline (speedup 1.0000x reference)
"""Bahdanau-attention score kernel (softmax(v . tanh(W[h;enc]+b))) for 8 TRN2 cores.

Self-contained: hardcodes shapes B=32, S=2048, ENC2=600, DD=900.
Sharding: data-parallel over batch (4 batches/core), weights replicated.
"""

import os

import numpy as np

import concourse.bass as bass  # noqa: F401
import concourse.mybir as mybir
import concourse.tile as tile
from concourse import bacc
from concourse.bass_utils import run_bass_kernel_spmd
from concourse.masks import make_identity

F32 = mybir.dt.float32
F32R = mybir.dt.float32r
AF = mybir.ActivationFunctionType
ALU = mybir.AluOpType
AX = mybir.AxisListType

NCORES = 8
B, S, E2, DD = 32, 2048, 600, 900
IN_DIM = DD + E2            # 1500
BL = B // NCORES            # 4 batches per core
SROWS = BL * S              # 8192 s-rows per core
P = 128
TBLK = 4                    # s-tiles per block
BLK = P * TBLK              # 512
NBLK = SROWS // BLK         # 16
NCOL = SROWS // P           # 64 score columns
ECH = [(0, 128), (128, 128), (256, 128), (384, 128), (512, 88)]   # e chunks of 600
DCH = [(i * 128, 128) for i in range(7)] + [(896, 4)]             # d chunks of 900
NSP = [(0, 512), (512, 388)]                                      # N splits of 900

# debug bisection knobs (full kernel by default)
K_NBLK = int(os.environ.get("K_NBLK", NBLK))
K_SOFT = int(os.environ.get("K_SOFT", "1"))
K_INIT = int(os.environ.get("K_INIT", "1"))


def build():
    nc = bacc.Bacc("TRN2", target_bir_lowering=False)
    enc_ext = nc.dram_tensor("enc", [SROWS, E2], F32, kind="ExternalInput")
    hid_ext = nc.dram_tensor("hid", [BL, DD], F32, kind="ExternalInput")
    w_ext = nc.dram_tensor("attn_w", [DD, IN_DIM], F32, kind="ExternalInput")
    b_ext = nc.dram_tensor("attn_b", [1, DD], F32, kind="ExternalInput")
    v_ext = nc.dram_tensor("v", [1, DD], F32, kind="ExternalInput")
    out_ext = nc.dram_tensor("out", [BL, S], F32, kind="ExternalOutput")
    scr_dram = nc.dram_tensor("scr", [1, BL], F32)  # softmax bcast scratch

    with tile.TileContext(nc) as tc:
        with (
            tc.tile_pool(name="stat", bufs=1) as stat,
            tc.tile_pool(name="ps_t", bufs=4, space="PSUM") as ps_t,
            tc.tile_pool(name="ps_e", bufs=2, space="PSUM") as ps_e,
        ):
            # ---------------- constants ----------------
            ident_f = stat.tile([P, P], F32)
            make_identity(nc, ident_f[:, :])
            ones_f = stat.tile([1, P], F32)
            nc.gpsimd.memset(ones_f[:, :], 1.0)
            ones_r = stat.tile([1, P], F32R)
            nc.scalar.copy(ones_r[:, :], ones_f[:, :])  # ACT op 1 (waits Pool)

            v_rep = stat.tile([P, DD], F32)
            nc.sync.dma_start(out=v_rep[:, :], in_=v_ext.ap().partition_broadcast(P))
            b_rep = stat.tile([BL, DD], F32)
            nc.sync.dma_start(out=b_rep[:, :], in_=b_ext.ap().partition_broadcast(BL))

            hb_flat = stat.tile([1, BL * DD], F32R)
            rhs = [
                stat.tile([ec, DD], F32R, tag=f"rhs{c}", name=f"rhs{c}")
                for c, (_, ec) in enumerate(ECH)
            ]
            scores = stat.tile([P, NCOL], F32)
            dve_scr = stat.tile([1, 4], F32)

            # DVE primes: absorb DMA sems for tiles DVE will read later
            nc.vector.tensor_copy(out=dve_scr[0:1, 0:1], in_=v_rep[0:1, 0:1])
            nc.vector.tensor_copy(out=dve_scr[0:1, 1:2], in_=b_rep[0:1, 0:1])

            # PE prime: observe ACT sem (identity not needed in f32r here;
            # first PE op that waits ACT covers ones_r/ident copies)
            pr1 = ps_t.tile([P, BLK], F32, tag="tp")
            nc.tensor.transpose(pr1[0:P, 0:P], ident_f[:, :], ident_f[:, :])

            # ---------------- init: weights / h_proj ----------------
            if not K_INIT:
                return nc
            with tc.tile_pool(name="init", bufs=1) as initp:
                w_rows = []
                for j, (os_, oc) in enumerate(DCH):
                    wj = initp.tile([oc, IN_DIM], F32, tag=f"w{j}")
                    nc.sync.dma_start(out=wj[:, :], in_=w_ext.ap()[os_:os_ + oc, :])
                    w_rows.append(wj)
                hid_stage = initp.tile([BL, DD], F32)
                nc.sync.dma_start(out=hid_stage[:, :], in_=hid_ext.ap())

                # WeT chunks -> rhs[c] (f32r)
                for c, (es, ec) in enumerate(ECH):
                    psw = ps_e.tile([P, DD], F32, tag="ep")
                    for j, (os_, oc) in enumerate(DCH):
                        nc.tensor.transpose(
                            psw[0:ec, os_:os_ + oc],
                            w_rows[j][:, DD + es:DD + es + ec],
                            ident_f[0:oc, 0:oc],
                        )
                    nc.scalar.copy(rhs[c][:, :], psw[0:ec, 0:DD])

                # WhT chunks + hidden^T chunks + h_proj matmuls
                hp = ps_e.tile([BL, DD], F32, tag="ep")
                for c, (ds, dc) in enumerate(DCH):
                    whT = initp.tile([dc, DD], F32R, tag=f"whT{c}")
                    for half, (no, nn) in enumerate(NSP):
                        psw = ps_t.tile([P, BLK], F32, tag="tp", name=f"psw{c}_{half}")
                        for j, (os_, oc) in enumerate(DCH):
                            if os_ >= no and os_ < no + nn:
                                nc.tensor.transpose(
                                    psw[0:dc, os_ - no:os_ - no + oc],
                                    w_rows[j][:, ds:ds + dc],
                                    ident_f[0:oc, 0:oc],
                                )
                        nc.scalar.copy(whT[:, no:no + nn], psw[0:dc, 0:nn])

                    psh = ps_t.tile([P, BLK], F32, tag="tp")
                    nc.tensor.transpose(
                        psh[0:dc, 0:BL], hid_stage[:, ds:ds + dc], ident_f[0:BL, 0:BL]
                    )
                    hidT = initp.tile([dc, BL], F32R, tag=f"hidT{c}")
                    nc.scalar.copy(hidT[:, :], psh[0:dc, 0:BL])

                    for (no, nn) in NSP:
                        nc.tensor.matmul(
                            hp[:, no:no + nn], hidT[:, :], whT[:, no:no + nn],
                            start=(c == 0), stop=(c == len(DCH) - 1),
                        )

                # hb = h_proj + attn_b  -> flat f32r row [1, BL*DD]
                hb_stage = initp.tile([BL, DD], F32)
                nc.vector.tensor_add(hb_stage[:, :], hp[:, :], b_rep[:, :])
                nc.gpsimd.dma_start(out=hb_flat[:, :], in_=hb_stage[:, :])

                # PE prime for hb_flat (SWDGE sem), 1 wait
                pr2 = ps_t.tile([P, BLK], F32, tag="tp")
                nc.tensor.matmul(
                    pr2[0:1, 0:512], ones_r[:, 0:1], hb_flat[:, 0:512],
                    start=True, stop=True,
                )

            # ---------------- main loop ----------------
            with (
                tc.tile_pool(name="encp", bufs=3) as encp,
                tc.tile_pool(name="etp", bufs=2) as etp,
                tc.tile_pool(name="zp", bufs=3) as zp,
                tc.tile_pool(name="jp", bufs=2) as jp,
            ):
                for k in range(K_NBLK):
                    bidx = k // (NBLK // BL)
                    enc_t = encp.tile([P, TBLK, E2], F32, tag="enc")
                    nc.sync.dma_start(
                        out=enc_t[:, :, :],
                        in_=enc_ext.ap()[k * BLK:(k + 1) * BLK, :].rearrange(
                            "(t p) e -> p t e", p=P
                        ),
                    )
                    encT = []
                    for c, (es, ec) in enumerate(ECH):
                        pst = ps_t.tile([P, BLK], F32, tag="tp")
                        for t in range(TBLK):
                            nc.tensor.transpose(
                                pst[0:ec, t * P:(t + 1) * P],
                                enc_t[:, t, es:es + ec],
                                ident_f[:, :],
                            )
                        et = etp.tile([ec, BLK], F32R, tag=f"et{c}")
                        nc.scalar.copy(et[:, :], pst[0:ec, :])
                        encT.append(et)

                    for t in range(TBLK):
                        eps = ps_e.tile([P, DD], F32, tag="ep")
                        for (no, nn) in NSP:
                            nc.tensor.matmul(
                                eps[:, no:no + nn],
                                ones_r[0:1, :],
                                hb_flat[0:1, bidx * DD + no: bidx * DD + no + nn],
                                start=True, stop=False,
                            )
                        for c, (es, ec) in enumerate(ECH):
                            for (no, nn) in NSP:
                                nc.tensor.matmul(
                                    eps[:, no:no + nn],
                                    encT[c][:, t * P:(t + 1) * P],
                                    rhs[c][:, no:no + nn],
                                    start=False, stop=(c == len(ECH) - 1),
                                )
                        z = zp.tile([P, DD], F32, tag="z")
                        nc.scalar.activation(z[:, :], eps[:, :], AF.Tanh)
                        junk = jp.tile([P, DD], F32, tag="junk")
                        nc.vector.tensor_mul(junk[:, :], z[:, :], v_rep[:, :])
                        col = TBLK * k + t
                        if t % 2 == 1:
                            dump = jp.tile([P, DD], F32, tag="dump")
                            nc.scalar.activation(
                                dump[:, :], junk[:, :], AF.Copy,
                                accum_out=scores[:, col:col + 1],
                            )
                        else:
                            nc.vector.tensor_reduce(
                                out=scores[:, col:col + 1], in_=junk[:, :],
                                axis=AX.X, op=ALU.add,
                            )

            # ---------------- softmax (no max-sub; scores are ~ +-50) ------
            if not K_SOFT:
                return nc
            ksn = int(os.environ.get("K_SOFTN", "13"))
            with tc.tile_pool(name="endp", bufs=1) as endp:
                # ACT absorber: observe DVE's final score writes
                absr = endp.tile([P, 1], F32)
                nc.scalar.copy(absr[:, :], scores[:, NCOL - 2:NCOL - 1])
                if ksn < 2:
                    return nc

                ps1 = ps_t.tile([P, BLK], F32, tag="tp")
                nc.tensor.transpose(ps1[0:NCOL, 0:P], scores[:, :], ident_f[:, :])
                if ksn < 3:
                    return nc
                scT = endp.tile([NCOL, P], F32)
                nc.scalar.copy(scT[:, :], ps1[0:NCOL, 0:P])
                if ksn < 4:
                    return nc

                e1 = endp.tile([NCOL, P], F32)
                rs = endp.tile([NCOL, 1], F32)
                nc.scalar.activation(e1[:, :], scT[:, :], AF.Exp, accum_out=rs[:, :])
                if ksn < 5:
                    return nc

                ps2 = ps_t.tile([P, BLK], F32, tag="tp")
                nc.tensor.transpose(ps2[0:1, 0:NCOL], rs[:, :], ident_f[0:NCOL, 0:NCOL])
                if ksn < 6:
                    return nc
                rsT = endp.tile([1, NCOL], F32)
                nc.scalar.copy(rsT[:, :], ps2[0:1, 0:NCOL])
                if ksn < 7:
                    return nc

                rb = endp.tile([1, BL], F32)
                nc.vector.tensor_reduce(
                    out=rb[:, :],
                    in_=rsT[0:1, :].rearrange("p (b t) -> p b t", b=BL),
                    axis=AX.X, op=ALU.add,
                )
                if ksn < 8:
                    return nc
                rbi = endp.tile([1, BL], F32)
                nc.vector.reciprocal(rbi[:, :], rb[:, :])
                if ksn < 9:
                    return nc
                nc.sync.dma_start(out=scr_dram.ap(), in_=rbi[:, :])
                if ksn < 10:
                    return nc
                rfac = endp.tile([NCOL, 1], F32)
                nbt = NCOL // BL   # 16
                for bb in range(BL):
                    nc.sync.dma_start(
                        out=rfac[bb * nbt:(bb + 1) * nbt, 0:1],
                        in_=scr_dram.ap()[0:1, bb:bb + 1].partition_broadcast(nbt),
                    )
                if ksn < 11:
                    return nc
                # DVE prime on rfac
                nc.vector.tensor_copy(out=dve_scr[0:1, 2:3], in_=rfac[0:1, 0:1])
                if ksn < 12:
                    return nc
                outf = endp.tile([NCOL, P], F32)
                nc.vector.tensor_scalar_mul(outf[:, :], e1[:, :], rfac[:, 0:1])
                if ksn < 13:
                    return nc
                nc.sync.dma_start(
                    out=out_ext.ap().rearrange("b (t p) -> (b t) p", p=P),
                    in_=outf[:, :],
                )
    return nc


_CACHE = {}


def _get_nc():
    if "nc" not in _CACHE:
        nc = build()
        nc.compile()
        _CACHE["nc"] = nc
    return _CACHE["nc"]


def make_in_maps(hidden, encoder_outputs, attn_W, attn_b, v):
    in_maps = []
    for c in range(NCORES):
        bs = slice(c * BL, (c + 1) * BL)
        in_maps.append({
            "enc": np.ascontiguousarray(
                np.asarray(encoder_outputs[bs], dtype=np.float32).reshape(SROWS, E2)
            ),
            "hid": np.ascontiguousarray(np.asarray(hidden[bs], dtype=np.float32)),
            "attn_w": np.ascontiguousarray(np.asarray(attn_W, dtype=np.float32)),
            "attn_b": np.asarray(attn_b, dtype=np.float32).reshape(1, DD),
            "v": np.asarray(v, dtype=np.float32).reshape(1, DD),
        })
    return in_maps


def run(in_maps, trace=False, **kw):
    nc = _get_nc()
    return run_bass_kernel_spmd(nc, in_maps, core_ids=list(range(NCORES)),
                                trace=trace, **kw)


def kernel(hidden, encoder_outputs, attn_W, attn_b, v):
    res = run(make_in_maps(hidden, encoder_outputs, attn_W, attn_b, v))
    out = np.concatenate([res.results[c]["out"] for c in range(NCORES)], axis=0)
    return np.ascontiguousarray(out, dtype=np.float32)


# revision 42
# speedup vs baseline: 1.2535x; 1.2535x over previous
"""Bahdanau-attention score kernel (softmax(v . tanh(W[h;enc]+b))) for 8 TRN2 cores.

Self-contained: hardcodes shapes B=32, S=2048, ENC2=600, DD=900.
Sharding: data-parallel over batch (4 batches/core), weights replicated.
"""

import contextlib
import os

import numpy as np

import concourse.bass as bass  # noqa: F401
import concourse.mybir as mybir
import concourse.tile as tile
from concourse import bacc
from concourse.bass_utils import run_bass_kernel_spmd
from concourse.masks import make_identity

F32 = mybir.dt.float32
F32R = mybir.dt.float32r
AF = mybir.ActivationFunctionType
ALU = mybir.AluOpType
AX = mybir.AxisListType

NCORES = 8
B, S, E2, DD = 32, 2048, 600, 900
IN_DIM = DD + E2            # 1500
BL = B // NCORES            # 4 batches per core
SROWS = BL * S              # 8192 s-rows per core
P = 128
TBLK = 4                    # s-tiles per block
BLK = P * TBLK              # 512
NBLK = SROWS // BLK         # 16
NCOL = SROWS // P           # 64 score columns
ECH = [(0, 128), (128, 128), (256, 128), (384, 128), (512, 88)]   # e chunks of 600
DCH = [(i * 128, 128) for i in range(7)] + [(896, 4)]             # d chunks of 900
NSP = [(0, 512), (512, 388)]                                      # N splits of 900
KA = 92          # chunk-4 contraction: 88 e-rows + 4 one-hot rows
NPRE = 2         # blocks whose transposes are emitted ahead of init

# debug bisection knobs (full kernel by default)
K_NBLK = int(os.environ.get("K_NBLK", NBLK))
K_SOFT = int(os.environ.get("K_SOFT", "1"))
K_INIT = int(os.environ.get("K_INIT", "1"))


def build():
    nc = bacc.Bacc("TRN2", target_bir_lowering=False)
    # f32r has identical bytes to f32 -- declaring inputs as f32r lets the
    # fast HWDGE DMA path (no dtype cast) feed the f32r matmuls directly
    enc_ext = nc.dram_tensor("enc", [SROWS, E2], F32R, kind="ExternalInput")
    hid_ext = nc.dram_tensor("hid", [BL, DD], F32, kind="ExternalInput")
    wt_ext = nc.dram_tensor("attn_wT", [IN_DIM, DD], F32R, kind="ExternalInput")
    b_ext = nc.dram_tensor("attn_b", [1, DD], F32, kind="ExternalInput")
    v_ext = nc.dram_tensor("v", [1, DD], F32, kind="ExternalInput")
    oh_ext = nc.dram_tensor("onehot", [BL * BL, BLK], F32R, kind="ExternalInput")
    out_ext = nc.dram_tensor("out", [BL, S], F32, kind="ExternalOutput")
    scr_dram = nc.dram_tensor("scr", [1, BL], F32)  # softmax bcast scratch

    with tile.TileContext(nc) as tc:
        with (
            tc.tile_pool(name="stat", bufs=1) as stat,
            tc.tile_pool(name="ps_t", bufs=4, space="PSUM") as ps_t,
            tc.tile_pool(name="ps_e", bufs=2, space="PSUM") as ps_e,
        ):
            # ---------------- constants ----------------
            ident_f = stat.tile([P, P], F32)
            make_identity(nc, ident_f[:, :])
            ident_r = stat.tile([P, P], F32R)
            nc.scalar.copy(ident_r[:, :], ident_f[:, :])

            enc_es = contextlib.ExitStack()
            encp = enc_es.enter_context(tc.tile_pool(name="encp", bufs=4))
            etp = enc_es.enter_context(tc.tile_pool(name="etp", bufs=3))

            # ---- DMA issue order: enc0, weights, enc1-3, one-hots, rest ----
            enc_tiles = {}

            def issue_enc(k):
                et_ = encp.tile([P, TBLK, E2], F32R, tag="enc", name=f"enc{k}")
                nc.sync.dma_start(
                    out=et_[:, :, :],
                    in_=enc_ext.ap()[k * BLK:(k + 1) * BLK, :].rearrange(
                        "(t p) e -> p t e", p=P
                    ),
                )
                enc_tiles[k] = et_

            if K_NBLK > 0:
                issue_enc(0)

            rhs_main = stat.tile([P, 4, DD], F32R)
            nc.sync.dma_start(
                out=rhs_main[:, :, :],
                in_=wt_ext.ap()[DD:DD + 512, :].rearrange("(c p) o -> p c o", p=P),
            )
            rhs4 = stat.tile([KA, DD], F32R)  # 88 WeT rows + 4 hb rows
            nc.sync.dma_start(out=rhs4[0:88, :], in_=wt_ext.ap()[DD + 512:IN_DIM, :])
            rhs = [rhs_main[:, c, :] for c in range(4)]

            whT_main = stat.tile([P, 8, DD], F32R)
            nc.sync.dma_start(
                out=whT_main[:, 0:7, :],
                in_=wt_ext.ap()[0:896, :].rearrange("(c p) o -> p c o", p=P),
            )
            nc.sync.dma_start(out=whT_main[0:4, 7, :], in_=wt_ext.ap()[896:DD, :])

            for k in range(1, min(4, K_NBLK)):
                issue_enc(k)

            # pre-write one-hot rows 88..91 into both slots of each per-batch
            # chunk-4 tag; the addresses persist across pool.tile reuse
            for b in range(BL):
                for i in range(2):
                    warm = etp.tile([KA, BLK], F32R, tag=f"et4_{b}",
                                    name=f"warm{b}_{i}", bufs=2)
                    nc.sync.dma_start(
                        out=warm[88:KA, :], in_=oh_ext.ap()[b * BL:(b + 1) * BL, :]
                    )

            v_rep = stat.tile([P, DD], F32)
            nc.sync.dma_start(out=v_rep[:, :], in_=v_ext.ap().partition_broadcast(P))
            b_rep = stat.tile([BL, DD], F32)
            nc.sync.dma_start(out=b_rep[:, :], in_=b_ext.ap().partition_broadcast(BL))
            hid_stage = stat.tile([BL, DD], F32)
            nc.sync.dma_start(out=hid_stage[:, :], in_=hid_ext.ap())

            scores = stat.tile([P, NCOL], F32)
            dve_scr = stat.tile([1, 4], F32)

            # DVE primes: absorb DMA sems for tiles DVE will read later
            nc.vector.tensor_copy(out=dve_scr[0:1, 0:1], in_=v_rep[0:1, 0:1])
            nc.vector.tensor_copy(out=dve_scr[0:1, 1:2], in_=b_rep[0:1, 0:1])

            # PE prime: observe ACT sem (ident_r) with one wait
            pr1 = ps_t.tile([P, BLK], F32R, tag="tp")
            nc.tensor.transpose(pr1[0:P, 0:P], ident_r[:, :], ident_r[:, :])

            # ---------------- per-block transposes + copies ----------------
            encT_blocks = {}

            def emit_transposes(k):
                bidx = k // (NBLK // BL)
                enc_t = enc_tiles[k]
                encT = []
                for c, (es, ec) in enumerate(ECH):
                    pst = ps_t.tile([P, BLK], F32R, tag="tp", name=f"pst{c}_{k}")
                    for t in range(TBLK):
                        nc.tensor.transpose(
                            pst[0:ec, t * P:(t + 1) * P],
                            enc_t[:, t, es:es + ec],
                            ident_r[:, :],
                        )
                    if c < 4:
                        et = etp.tile([ec, BLK], F32R, tag=f"et{c}",
                                      name=f"et{c}_{k}")
                    else:
                        # rows 88..91 hold the pre-written one-hot(batch)
                        et = etp.tile([KA, BLK], F32R, tag=f"et4_{bidx}",
                                      name=f"et4_{k}", bufs=2)
                    nc.scalar.copy(et[0:ec, :], pst[0:ec, :])
                    encT.append(et)
                encT_blocks[k] = encT

            if not K_INIT:
                return nc

            # software pipeline: first blocks' transposes ahead of init
            for k in range(min(NPRE, K_NBLK)):
                emit_transposes(k)

            # ---------------- init: h_proj ----------------
            with tc.tile_pool(name="init", bufs=1) as initp:
                # PE primes for the weight DMAs (one wait each)
                for nm, src in (("pm_r", rhs_main[:, 0, 0:P]),
                                ("pm_r4", rhs4[0:88, 0:P]),
                                ("pm_w", whT_main[:, 0, 0:P]),
                                ("pm_w2", whT_main[0:4, 7, 0:P])):
                    prt = ps_t.tile([P, BLK], F32R, tag="tp", name=f"ps_{nm}")
                    nc.tensor.transpose(
                        prt[0:P, 0:src.shape[0]],
                        src,
                        ident_r[0:src.shape[0], 0:src.shape[0]],
                    )

                # hidden^T chunks + h_proj matmuls
                hp = ps_e.tile([BL, DD], F32, tag="ep")
                for c, (ds, dc) in enumerate(DCH):
                    psh = ps_t.tile([P, BLK], F32, tag="tp")
                    nc.tensor.transpose(
                        psh[0:dc, 0:BL], hid_stage[:, ds:ds + dc],
                        ident_f[0:BL, 0:BL]
                    )
                    hidT = initp.tile([dc, BL], F32R, tag=f"hidT{c}")
                    nc.scalar.copy(hidT[:, :], psh[0:dc, 0:BL])

                    for (no, nn) in NSP:
                        nc.tensor.matmul(
                            hp[:, no:no + nn], hidT[:, :],
                            whT_main[0:dc, c, no:no + nn],
                            start=(c == 0), stop=(c == len(DCH) - 1),
                        )

                # hb = h_proj + attn_b -> rhs4 rows 88..91 (f32r, SWDGE cast)
                hb_stage = initp.tile([BL, DD], F32)
                nc.vector.tensor_add(hb_stage[:, :], hp[:, :], b_rep[:, :])
                nc.gpsimd.dma_start(out=rhs4[88:KA, :], in_=hb_stage[:, :])

                # PE prime for rhs4's hb rows (SWDGE sem), 1 wait
                pr2 = ps_t.tile([P, BLK], F32R, tag="tp")
                nc.tensor.transpose(
                    pr2[0:P, 0:KA], rhs4[0:KA, 0:P], ident_r[0:KA, 0:KA]
                )

            # ---------------- main loop ----------------
            with (
                tc.tile_pool(name="zp", bufs=4) as zp,
                tc.tile_pool(name="jp", bufs=2) as jp,
            ):
                for k in range(K_NBLK):
                    bidx = k // (NBLK // BL)
                    if k not in enc_tiles:
                        issue_enc(k)
                    if k not in encT_blocks:
                        emit_transposes(k)
                    encT = encT_blocks.pop(k)

                    for t in range(TBLK):
                        eps = ps_e.tile([P, DD], F32, tag="ep")
                        for c, (es, ec) in enumerate(ECH):
                            lhs = (encT[c][:, t * P:(t + 1) * P] if c < 4
                                   else encT[4][0:KA, t * P:(t + 1) * P])
                            rr = rhs[c] if c < 4 else rhs4
                            for (no, nn) in NSP:
                                nc.tensor.matmul(
                                    eps[:, no:no + nn],
                                    lhs,
                                    rr[:, no:no + nn],
                                    start=(c == 0), stop=(c == len(ECH) - 1),
                                )
                        z = zp.tile([P, DD], F32, tag="z")
                        nc.scalar.activation(z[:, :], eps[:, :], AF.Tanh)
                        junk = jp.tile([P, DD], F32, tag="junk")
                        nc.vector.tensor_mul(junk[:, :], z[:, :], v_rep[:, :])
                        col = TBLK * k + t
                        if t == 1:
                            # one ACT reduce per block keeps ACT's DVE clock
                            # fresh (z-slot release discipline)
                            dump = jp.tile([P, DD], F32, tag="dump")
                            nc.scalar.activation(
                                dump[:, :], junk[:, :], AF.Copy,
                                accum_out=scores[:, col:col + 1],
                            )
                        else:
                            nc.vector.tensor_reduce(
                                out=scores[:, col:col + 1], in_=junk[:, :],
                                axis=AX.X, op=ALU.add,
                            )
            enc_es.close()

            # ---------------- softmax (no max-sub; scores are ~ +-50) ------
            if not K_SOFT:
                return nc
            ksn = int(os.environ.get("K_SOFTN", "13"))
            with tc.tile_pool(name="endp", bufs=1) as endp:
                # ACT absorber: observe DVE's final score writes
                absr = endp.tile([P, 1], F32)
                nc.scalar.copy(absr[:, :], scores[:, NCOL - 1:NCOL])
                if ksn < 2:
                    return nc

                ps1 = ps_t.tile([P, BLK], F32, tag="tp")
                nc.tensor.transpose(ps1[0:NCOL, 0:P], scores[:, :], ident_f[:, :])
                if ksn < 3:
                    return nc
                scT = endp.tile([NCOL, P], F32)
                nc.scalar.copy(scT[:, :], ps1[0:NCOL, 0:P])
                if ksn < 4:
                    return nc

                e1 = endp.tile([NCOL, P], F32)
                rs = endp.tile([NCOL, 1], F32)
                nc.scalar.activation(e1[:, :], scT[:, :], AF.Exp, accum_out=rs[:, :])
                if ksn < 5:
                    return nc

                ps2 = ps_t.tile([P, BLK], F32, tag="tp")
                nc.tensor.transpose(ps2[0:1, 0:NCOL], rs[:, :],
                                    ident_f[0:NCOL, 0:NCOL])
                if ksn < 6:
                    return nc
                rsT = endp.tile([1, NCOL], F32)
                nc.scalar.copy(rsT[:, :], ps2[0:1, 0:NCOL])
                if ksn < 7:
                    return nc

                rb = endp.tile([1, BL], F32)
                nc.vector.tensor_reduce(
                    out=rb[:, :],
                    in_=rsT[0:1, :].rearrange("p (b t) -> p b t", b=BL),
                    axis=AX.X, op=ALU.add,
                )
                if ksn < 8:
                    return nc
                rbi = endp.tile([1, BL], F32)
                nc.vector.reciprocal(rbi[:, :], rb[:, :])
                if ksn < 9:
                    return nc
                nc.sync.dma_start(out=scr_dram.ap(), in_=rbi[:, :])
                if ksn < 10:
                    return nc
                rfac = endp.tile([NCOL, 1], F32)
                nbt = NCOL // BL   # 16
                for bb in range(BL):
                    nc.sync.dma_start(
                        out=rfac[bb * nbt:(bb + 1) * nbt, 0:1],
                        in_=scr_dram.ap()[0:1, bb:bb + 1].partition_broadcast(nbt),
                    )
                if ksn < 11:
                    return nc
                # DVE prime on rfac
                nc.vector.tensor_copy(out=dve_scr[0:1, 2:3], in_=rfac[0:1, 0:1])
                if ksn < 12:
                    return nc
                outf = endp.tile([NCOL, P], F32)
                nc.vector.tensor_scalar_mul(outf[:, :], e1[:, :], rfac[:, 0:1])
                if ksn < 13:
                    return nc
                nc.sync.dma_start(
                    out=out_ext.ap().rearrange("b (t p) -> (b t) p", p=P),
                    in_=outf[:, :],
                )
    return nc


_CACHE = {}


def _get_nc():
    if "nc" not in _CACHE:
        nc = build()
        nc.compile()
        _CACHE["nc"] = nc
    return _CACHE["nc"]


def make_in_maps(hidden, encoder_outputs, attn_W, attn_b, v):
    in_maps = []
    for c in range(NCORES):
        bs = slice(c * BL, (c + 1) * BL)
        in_maps.append({
            "enc": np.ascontiguousarray(
                np.asarray(encoder_outputs[bs], dtype=np.float32).reshape(SROWS, E2)
            ),
            "hid": np.ascontiguousarray(np.asarray(hidden[bs], dtype=np.float32)),
            "attn_wT": np.ascontiguousarray(np.asarray(attn_W, dtype=np.float32).T),
            "attn_b": np.asarray(attn_b, dtype=np.float32).reshape(1, DD),
            "v": np.asarray(v, dtype=np.float32).reshape(1, DD),
            "onehot": np.ascontiguousarray(
                np.repeat(np.eye(BL, dtype=np.float32).reshape(BL * BL, 1),
                          BLK, axis=1)
            ),
        })
    return in_maps


def run(in_maps, trace=False, **kw):
    nc = _get_nc()
    return run_bass_kernel_spmd(nc, in_maps, core_ids=list(range(NCORES)),
                                trace=trace, **kw)


def kernel(hidden, encoder_outputs, attn_W, attn_b, v):
    res = run(make_in_maps(hidden, encoder_outputs, attn_W, attn_b, v))
    out = np.concatenate([res.results[c]["out"] for c in range(NCORES)], axis=0)
    return np.ascontiguousarray(out, dtype=np.float32)
